# revision 41
# baseline (speedup 1.0000x reference)
"""Trainium2 Bass kernel: BigramHashEmbedding (hash -> embed gather -> proj -> scale).

Computation (per batch row, one NeuronCore per row, 8 rows total):
    h[0]  = 10239
    h[j]  = (36313*t[j] ^ 27191*t[j-1]) % 10239          (int32, j >= 1)
    e     = embed_weight[h]                               [S, 128] gather
    out   = (e @ proj_weight.T) * scale                   [S, 512]

Host-side prep (in kernel()): embed table cast to fp16, proj pre-transposed,
pre-scaled and cast to fp16, output produced in fp16 and upcast on host.

Device strategy per core (S = 8192 tokens):
  * tokens live in SBUF as [128, 64] int32 (partition p holds tokens
    64p..64p+63) -- one contiguous DMA, no replication.  The bigram hash runs
    once per token on DVE/ACT with fp32-exact arithmetic (split multiplies,
    limb-decomposed mod as in the original scheme).  The cross-partition
    boundary token t[64p-1] is produced by two tiny PE transposes (no DMA).
  * dma_gather consumes indices from a 16-partition-wrapped, 8x-replicated
    int16 tile: slot k reads idx[k%16, k//16] and lands at SBUF partition
    k%128, block k//128.  We want slot k == token k (so each 128-slot block
    is 128 consecutive tokens and output writes are fully contiguous), which
    needs idx[16i+g, j] = h[16j+g].  The hash tile H[p, c] = h[64p+c] is
    sliced into four [128,16] slabs (c = 16b..16b+15), free-dim replicated
    8x and PE-transposed: out[16i+g, m] = H[m, 16b+g] = h[64m+16b+g], which
    is exactly idx column 4m+b -- a strided DVE copy into the idx tile.
  * eight dma_gathers (1024 rows each, 4 SWDGE queues round-robin in
    emission order) fetch fp16 embed rows into g_sb [128, 64, 128].
  * per 128-token block: PE transpose (fp16 identity) -> PSUM -> eT in SBUF,
    then PE matmul eT.T @ projT -> PSUM f32 -> SBUF fp16 (ACT/DVE
    alternating), grouped GRP blocks per contiguous output DMA.
"""

from contextlib import ExitStack

import numpy as np

import concourse.bacc as bacc
import concourse.bass as bass
import concourse.mybir as mybir
import concourse.tile as tile
from concourse.bass_utils import run_bass_kernel_spmd
from concourse.masks import make_identity

AL = mybir.AluOpType
F32 = mybir.dt.float32
F16 = mybir.dt.float16
I32 = mybir.dt.int32
I16 = mybir.dt.int16

B = 8           # batch rows == cores
S = 8192        # tokens per core
V = 10240       # hash table rows
D = 128         # embed dim
M = 512         # model dim
P = 128
MOD = 10239     # hash modulus (HASH_SIZE - 1)
SPT = S // P    # tokens per partition = 64
NG = 8          # sub-gathers
TPG = S // NG   # tokens per gather = 1024
CPG = TPG // 16  # idx columns per gather = 64
NB = S // P     # 128-token blocks = 64
BPG = NB // NG  # blocks per gather = 8
NSLAB = SPT // 16  # idx transpose slabs = 4

# 36313 = 141*256 + 217 ; 27191 = 106*256 + 55
A_HI, A_LO = 141, 217
B_HI, B_LO = 106, 55
C21 = 8396      # 2^21 mod 10239
INV_M = 1.0 / MOD

USE_ACT_MUL = True   # run the big hash multiplies on the Scalar (ACT) engine
N_QUEUES = 4         # SWDGE queues
LAG = 4              # transpose runs LAG blocks ahead of the matmul
GRP = 4              # blocks per output DMA (512 contiguous rows)

# Hybrid split: the last FUSED_BLOCKS 128-token blocks gather 512-wide rows
# from the host-fused (table @ projT) table straight to the output (no PE /
# PSUM work at all); the rest go through the 128-wide gather + matmul path.
# Balances SWDGE descriptor time (17ns/KB-row vs 12ns/256B-row) against the
# PE+ACT+DVE cost of the matmul pipeline (~1.6us per block).
FUSED_BLOCKS = 40
MM_BLOCKS = NB - FUSED_BLOCKS          # 24
NGF = FUSED_BLOCKS // 4                # fused gathers (4 blocks each)
NGM = 6                                # mm gathers
TPGM = MM_BLOCKS * P // NGM            # 512 slots per mm gather
BPGM = TPGM // P                       # 4 blocks per mm gather


def _mul(nc, out, in_, const):
    if USE_ACT_MUL:
        nc.scalar.mul(out, in_, float(const))
    else:
        nc.vector.tensor_scalar_mul(out, in_, float(const))


def _hash(nc, tmp, r, toks_v, tm1):
    """Compute r[p, c] = hash(t[64p+c]) for the [128, SPT] token tile.

    toks_v: [128, SPT, W] int32 view of the token tile (lo word at w=0).
    tm1:    [128, 1] f32, t[64p - 1] per partition (0 at p==0, masked later).
    """
    n = SPT
    tcur = toks_v[:, 0:n, 0:1]
    p1 = tmp.tile([P, n], I32, tag="p1")
    p2 = tmp.tile([P, n], I32, tag="p2")
    q1 = tmp.tile([P, n], I32, tag="q1")
    q2 = tmp.tile([P, n], I32, tag="q2")
    # whole chain on DVE back-to-back (no cross-engine sem round trips);
    # only the tm1-dependent column-0 products go to ACT in parallel
    tprev = toks_v[:, 0:n - 1, 0:1]
    nc.vector.tensor_scalar_mul(q1[:, 1:n], tprev, float(B_LO))
    nc.vector.tensor_scalar_mul(q2[:, 1:n], tprev, float(B_HI))
    nc.vector.tensor_scalar_mul(p1[:], tcur, float(A_LO))
    nc.vector.tensor_scalar_mul(p2[:], tcur, float(A_HI))
    nc.scalar.mul(q1[:, 0:1], tm1[:], float(B_LO))
    nc.scalar.mul(q2[:, 0:1], tm1[:], float(B_HI))

    # A>>8 = p2 + (p1>>8);  B>>8 = q2 + (q1>>8)   (both < 2^23, exact)
    # (the compiler rejects bitwise op0 fused with arith op1, so shift and
    # add are separate instructions)
    ah = tmp.tile([P, n], I32, tag="ah")
    bh = tmp.tile([P, n], I32, tag="bh")
    t1 = tmp.tile([P, n], I32, tag="t1")
    nc.vector.tensor_single_scalar(t1[:], p1[:], 8, op=AL.logical_shift_right)
    nc.vector.tensor_add(ah[:], t1[:], p2[:])
    nc.vector.tensor_single_scalar(t1[:], q1[:], 8, op=AL.logical_shift_right)
    nc.vector.tensor_add(bh[:], t1[:], q2[:])
    # X>>8 and X low byte (in low 8 bits of xl)
    xh = tmp.tile([P, n], I32, tag="xh")
    xl = tmp.tile([P, n], I32, tag="xl")
    nc.vector.tensor_tensor(xh[:], ah[:], bh[:], op=AL.bitwise_xor)
    nc.vector.tensor_tensor(xl[:], p1[:], q1[:], op=AL.bitwise_xor)

    # y = (xh>>13)*8396 + ((xh & 8191) << 8) + (xl & 255)   ( < 2^24 )
    w1 = tmp.tile([P, n], I32, tag="w1")
    w2 = tmp.tile([P, n], I32, tag="w2")
    nc.vector.tensor_single_scalar(w1[:], xh[:], 13, op=AL.logical_shift_right)
    nc.vector.tensor_scalar_mul(w1[:], w1[:], float(C21))
    nc.vector.tensor_scalar(w2[:], xh[:], 8191, 8,
                            op0=AL.bitwise_and, op1=AL.logical_shift_left)
    w3 = tmp.tile([P, n], I32, tag="w3")
    nc.vector.tensor_add(w3[:], w1[:], w2[:])
    y = tmp.tile([P, n], I32, tag="y")
    nc.vector.tensor_single_scalar(y[:], xl[:], 255, op=AL.bitwise_and)
    nc.vector.tensor_add(y[:], y[:], w3[:])

    # r' = y - (rne(y/m) - 1)*m = (y mod m) + m or + 2m -- always in (0, 2m),
    # so gathering from the doubled table needs no sign fixup at all.
    # (bias folds the -1 into the existing ACT multiply; x-1 is fp32-exact.)
    qt = tmp.tile([P, n], I32, tag="qt")
    nc.vector.tensor_scalar(qt[:], y[:], INV_M, -1.0,
                            op0=AL.mult, op1=AL.add)
    nc.vector.scalar_tensor_tensor(r[:], qt[:], -float(MOD), y[:],
                                   op0=AL.mult, op1=AL.add)
    # token 0: h = MOD, stored at doubled-table row 2m (its dedicated slot)
    nc.vector.tensor_scalar(r[0:1, 0:1], r[0:1, 0:1], 0.0, float(2 * MOD),
                            op0=AL.mult, op1=AL.add)


def body(ctx: ExitStack, tc: tile.TileContext, out_ap, tok_ap, table_ap,
         ftable_ap, projT_ap, W: int):
    """Emit the per-core kernel. tok_ap is int32 [S*W] (W=2 -> int64 lo/hi)."""
    nc = tc.nc

    const = ctx.enter_context(tc.tile_pool(name="const", bufs=1))
    tmp = ctx.enter_context(tc.tile_pool(name="tmp", bufs=2))
    gpool = ctx.enter_context(tc.tile_pool(name="gpool", bufs=1))
    et_pool = ctx.enter_context(tc.tile_pool(name="et", bufs=LAG + 2))
    o_pool = ctx.enter_context(tc.tile_pool(name="osb", bufs=3))

    # ---- tokens: [128, SPT*W] contiguous, partition p = tokens 64p.. ----
    FW = SPT * W
    toks = const.tile([P, FW], I32)
    nc.sync.dma_start(toks[:], tok_ap.rearrange("(p f) -> p f", p=P))
    toks_v = toks.rearrange("p (s w) -> p s w", w=W)

    # ---- constants: identity (f32 + f16), projT ----
    ident_f = const.tile([P, P], F32)
    make_identity(nc, ident_f[:])
    ident_h = const.tile([P, P], F16)
    nc.vector.tensor_copy(ident_h[:], ident_f[:])
    projT = const.tile([P, M], F16)
    nc.sync.dma_start(projT[:], projT_ap)

    ps_set = tc.alloc_tile_pool(name="ps_set", bufs=2, space="PSUM")

    # ---- boundary token t[64p-1] via two PE transposes ----
    tl_f = tmp.tile([P, 1], F32, tag="tl")
    nc.vector.tensor_copy(tl_f[:], toks_v[:, SPT - 1:SPT, 0:1])
    ps_r = ps_set.tile([P, P], F32, space="PSUM", tag="ps_r")
    nc.tensor.matmul(ps_r[0:1, :], lhsT=tl_f[:], rhs=ident_f[:],
                     start=True, stop=True)
    row = tmp.tile([1, P], F32, tag="row")
    nc.gpsimd.memset(row[:], 0)
    nc.vector.tensor_copy(row[0:1, 1:P], ps_r[0:1, 0:P - 1])
    ps_r2 = ps_set.tile([P, 1], F32, space="PSUM", tag="ps_r2")
    nc.tensor.matmul(ps_r2[:], lhsT=row[:], rhs=ident_f[0:1, 0:1],
                     start=True, stop=True)
    tm1 = const.tile([P, 1], F32)
    nc.vector.tensor_copy(tm1[:], ps_r2[:])

    # ---- hash -> idx (16-wrapped, replicated, token-ordered slots) ----
    h_f = const.tile([P, SPT], F32)
    _hash(nc, tmp, h_f, toks_v, tm1)

    idx = const.tile([P, S // 16], I16)
    idxv = idx.rearrange("p (m four) -> p m four", four=NSLAB)
    for b in range(NSLAB):
        h_rep = tmp.tile([P, P], F32, tag="h_rep")
        nc.vector.tensor_copy(
            h_rep[:], h_f[:, None, 16 * b:16 * (b + 1)].broadcast_to([P, 8, 16]))
        ps_i = ps_set.tile([P, P], F32, space="PSUM", tag="ps_i")
        nc.tensor.matmul(ps_i[:], lhsT=h_rep[:], rhs=ident_f[:],
                         start=True, stop=True)
        nc.vector.tensor_copy(idxv[:, :, b:b + 1], ps_i[:])

    # ---- gathers: slot k = token k; block b = tokens 128b..128b+127 ----
    # mm path: blocks 0..MM_BLOCKS-1 (128-wide rows); fused path: the rest
    # (512-wide pre-projected rows, straight to the output DMA).
    g_sb = gpool.tile([P, MM_BLOCKS, P], F16)
    gf_sb = gpool.tile([P, FUSED_BLOCKS, M], F16)
    swdge_i = 0
    for g in range(NGM):
        nc.gpsimd.dma_gather(
            g_sb[:, BPGM * g:BPGM * (g + 1), :],
            table_ap,
            idx[:, (TPGM // 16) * g:(TPGM // 16) * (g + 1)],
            num_idxs=TPGM,
            num_idxs_reg=TPGM,
            elem_size=D,
            single_packet=False,
            queue_num=swdge_i % N_QUEUES,
        )
        swdge_i += 1
    ovf = out_ap.rearrange("(c p) m -> p c m", p=P)
    fcol0 = (MM_BLOCKS * P) // 16  # first idx column of the fused range
    for g in range(NGF):
        blk = 4 * g
        nc.gpsimd.dma_gather(
            gf_sb[:, blk:blk + 4, :],
            ftable_ap,
            idx[:, fcol0 + 32 * g:fcol0 + 32 * (g + 1)],
            num_idxs=512,
            num_idxs_reg=512,
            elem_size=M,
            single_packet=False,
            queue_num=swdge_i % N_QUEUES,
        )
        swdge_i += 1
        oeng = nc.scalar if g % 2 == 0 else nc.sync
        oeng.dma_start(
            ovf[:, MM_BLOCKS + blk:MM_BLOCKS + blk + 4, :],
            gf_sb[:, blk:blk + 4, :])

    ps_set.release()
    ps_small = ctx.enter_context(tc.tile_pool(name="ps_small", bufs=4,
                                              space="PSUM"))
    ps_big = ctx.enter_context(tc.tile_pool(name="ps_big", bufs=4,
                                            space="PSUM"))

    ov = out_ap.rearrange("(gi gb p) m -> gi p gb m", gb=GRP, p=P)
    ets = {}
    o4s = {}

    def emit_trans(b):
        et = et_pool.tile([P, P], F16, tag="et", name=f"et{b}")
        ps_et = ps_small.tile([P, P], F16, space="PSUM",
                              tag="ps_et", name=f"ps_et{b}")
        nc.tensor.transpose(ps_et[:], g_sb[:, b, :], ident_h[:])
        if b % 2 == 0:
            nc.scalar.copy(et[:], ps_et[:])
        else:
            nc.vector.tensor_copy(et[:], ps_et[:])
        ets[b] = et

    def emit_mm(b):
        et = ets.pop(b)
        gi, gb = divmod(b, GRP)
        if gb == 0:
            o4s[gi] = o_pool.tile([P, GRP * M], F16, tag="o_sb",
                                  name=f"o4_{gi}")
        o4 = o4s[gi]
        ps_o = ps_big.tile([P, M], F32, space="PSUM", tag="ps_o",
                           name=f"ps_o{b}")
        nc.tensor.matmul(ps_o[:], lhsT=et[:], rhs=projT[:],
                         start=True, stop=True)
        dst = o4[:, M * gb:M * (gb + 1)]
        # opposite parity from the eT copies so each block uses both engines
        if b % 2 == 0:
            nc.vector.tensor_copy(dst, ps_o[:])
        else:
            nc.scalar.copy(dst, ps_o[:])
        if gb == GRP - 1:
            nc.sync.dma_start(ov[gi], o4[:])
            del o4s[gi]

    for b in range(MM_BLOCKS):
        emit_trans(b)
        if b >= LAG:
            emit_mm(b - LAG)
    for b in range(MM_BLOCKS - LAG, MM_BLOCKS):
        emit_mm(b)


_CACHE: dict = {}


def _build(W: int):
    if W in _CACHE:
        return _CACHE[W]
    nc = bacc.Bacc("TRN2", target_bir_lowering=False, debug=False,
                   num_swdge_queues=N_QUEUES, dynamic_dma_scratch_size=65536)
    tok = nc.dram_tensor("token_ids", [S * W], I32, kind="ExternalInput").ap()
    table = nc.dram_tensor("table", [2 * MOD + 1, D], F16,
                           kind="ExternalInput").ap()
    ftable = nc.dram_tensor("ftable", [2 * MOD + 1, M], F16,
                            kind="ExternalInput").ap()
    projT = nc.dram_tensor("projT", [D, M], F16, kind="ExternalInput").ap()
    out = nc.dram_tensor("out", [S, M], F16, kind="ExternalOutput").ap()
    with tile.TileContext(nc) as tc:
        with ExitStack() as ctx:
            body(ctx, tc, out, tok, table, ftable, projT, W)
    nc.compile()
    _CACHE[W] = nc
    return nc


def _prep(token_ids: np.ndarray, embed_weight: np.ndarray,
          proj_weight: np.ndarray, scale: np.ndarray):
    token_ids = np.ascontiguousarray(token_ids)
    assert token_ids.shape == (B, S), token_ids.shape
    W = 2 if token_ids.dtype.itemsize == 8 else 1
    tok32 = token_ids.view(np.int32).reshape(B, S * W)
    t16 = np.asarray(embed_weight, dtype=np.float32).astype(np.float16)
    # index r' = (h mod m) + m or + 2m lands on the right row iff the copy
    # period is m = MOD (not V); row 2m holds the h == MOD head embedding
    table = np.ascontiguousarray(
        np.concatenate([t16[:MOD], t16[:MOD], t16[MOD:MOD + 1]], axis=0))
    sc = float(np.asarray(scale, dtype=np.float32).reshape(()))
    projT = np.ascontiguousarray(
        (np.asarray(proj_weight, dtype=np.float32).T * sc).astype(np.float16))
    # pre-projected table for the fused gather path (same doubled layout)
    ftable = np.ascontiguousarray(
        (table.astype(np.float32) @ projT.astype(np.float32))
        .astype(np.float16))
    in_maps = [
        {
            "token_ids": np.ascontiguousarray(tok32[i]),
            "table": table,
            "ftable": ftable,
            "projT": projT,
        }
        for i in range(B)
    ]
    return W, in_maps


def kernel(token_ids: np.ndarray, embed_weight: np.ndarray,
           proj_weight: np.ndarray, scale: np.ndarray) -> np.ndarray:
    W, in_maps = _prep(token_ids, embed_weight, proj_weight, scale)
    nc = _build(W)
    res = run_bass_kernel_spmd(nc, in_maps, core_ids=list(range(B)))
    return np.stack([r["out"] for r in res.results], axis=0).astype(np.float32)
